# revision 8
# baseline (speedup 1.0000x reference)
"""EntityAttentionLayer on 8 Trainium2 NeuronCores (Bass/Tile).

Reference computation (per batch b of 1024):
    qkv = entities @ W_in.T            # [128 ents, 3*512]
    q (first 32 ents), k, v -> 8 heads x 64
    logits = q k^T / 8, masked by pre_mask (True = masked out)
    w = softmax(logits), fully-masked rows -> 0
    out = (w v) @ W_out.T + b_out, zeroed where post_mask

Sharding: data-parallel over batch, 128 batches per core.

Per-core kernel layout choices:
  - All big matmuls run as fp32r (TF32-class, full PE rate at N>=256).
  - QKV projections computed feature-major (q^T, k^T: [feat, tok]) for the
    logits matmuls; V computed token-major ([ent, feat]) for attn@v.
  - logits psum tile is [(head//2)*32+agent, batch, head%2, ent] so all 8
    heads of one batch run as one 8-slot tile_position group on the PE.
  - softmax over the free (ent) axis: fused mask-mul+row-sum
    (tensor_tensor_reduce), reciprocal_approx_fast, per-partition scale.
  - w is PE-transposed per (batch, head-parity) so attn@v contracts over
    entities; attn output lands feature-major, feeding the out-projection
    directly; final output is stored [out_feat, batch, agent] and
    untransposed on the host.
"""
import sys

sys.path.insert(0, "/opt/trn_rl_repo")

import numpy as np

BS, NE, IN_DIM = 1024, 128, 512
EMBED, OUT_DIM = 512, 512
N_HEADS, N_AGENTS = 8, 32
HEAD_DIM = EMBED // N_HEADS  # 64
N_CORES = 8


def build_nc(b_core: int):
    """Build the per-core Bass program for b_core batches (b_core % 8 == 0)."""
    import concourse.tile as tile
    from concourse import bacc, mybir
    from concourse.masks import make_identity

    F32 = mybir.dt.float32
    F32R = mybir.dt.float32r
    BF16 = mybir.dt.bfloat16
    Exp = mybir.ActivationFunctionType.Exp
    mult = mybir.AluOpType.mult
    add = mybir.AluOpType.add

    assert b_core % 8 == 0
    n_iter = b_core // 8

    nc = bacc.Bacc("TRN2", target_bir_lowering=False, debug=False)

    # DRAM I/O (per core). Strides in elements, C-order host arrays.
    xt_d = nc.declare_dram_parameter("xt", [b_core, IN_DIM, NE], F32R, isOutput=False)
    xta_d = nc.declare_dram_parameter("xta", [4, 128, b_core, N_AGENTS], F32R, isOutput=False)
    wi_d = nc.declare_dram_parameter("wi", [IN_DIM, 3 * EMBED], F32R, isOutput=False)
    wo_d = nc.declare_dram_parameter("wo", [EMBED, OUT_DIM], F32R, isOutput=False)
    keep_d = nc.declare_dram_parameter("keep", [b_core, N_AGENTS, NE], F32, isOutput=False)
    pkeep_d = nc.declare_dram_parameter("pkeep", [b_core, N_AGENTS], F32, isOutput=False)
    bias_d = nc.declare_dram_parameter("bias", [OUT_DIM], F32, isOutput=False)
    out_d = nc.declare_dram_parameter("out", [OUT_DIM, b_core, N_AGENTS], F32, isOutput=True)

    import concourse.bass as bass

    AP = bass.AP

    def dram_ap(handle, offset, ap):
        base = handle[:]
        return AP(tensor=base.tensor, offset=offset, ap=ap)

    with tile.TileContext(nc) as tc:
        with (
            tc.tile_pool(name="const", bufs=1) as constp,
            tc.tile_pool(name="ins", bufs=2) as insp,
            tc.tile_pool(name="mid", bufs=1) as midp,
            tc.tile_pool(name="attn", bufs=2) as attnp,
            tc.tile_pool(name="outs", bufs=2) as outsp,
            tc.tile_pool(name="ps_mm", bufs=2, space="PSUM") as ps_mm,
            tc.tile_pool(name="ps_lg", bufs=1, space="PSUM") as ps_lg,
            tc.tile_pool(name="ps_wt", bufs=1, space="PSUM") as ps_wt,
            tc.tile_pool(name="ps_at", bufs=1, space="PSUM") as ps_at,
            tc.tile_pool(name="ps_op", bufs=1, space="PSUM") as ps_op,
        ):
            # ---- constants (loaded once) ----
            wi_sb = constp.tile([128, 4, 3 * EMBED], F32R)  # [in%128, in//128, feat]
            nc.sync.dma_start(
                out=wi_sb,
                in_=dram_ap(wi_d, 0, [[3 * EMBED, 128], [128 * 3 * EMBED, 4], [1, 3 * EMBED]]),
            )
            wo_sb = constp.tile([128, 4, OUT_DIM], F32R)
            nc.sync.dma_start(
                out=wo_sb,
                in_=dram_ap(wo_d, 0, [[OUT_DIM, 128], [128 * OUT_DIM, 4], [1, OUT_DIM]]),
            )
            bias_sb = constp.tile([128, 4], F32)
            nc.sync.dma_start(out=bias_sb, in_=dram_ap(bias_d, 0, [[1, 128], [128, 4]]))
            ident = constp.tile([128, 128], BF16)
            make_identity(nc, ident)

            for it in range(n_iter):
                b0 = it * 8
                # ---- input DMAs for this iter (8 batches) ----
                xt_sb = insp.tile([128, 4, 8, NE], F32R)  # [in%128, ki, b, e]
                for ki in range(4):
                    nc.sync.dma_start(
                        out=xt_sb[:, ki, :, :],
                        in_=dram_ap(
                            xt_d,
                            b0 * IN_DIM * NE + ki * 128 * NE,
                            [[NE, 128], [IN_DIM * NE, 8], [1, NE]],
                        ),
                    )
                xta_sb = insp.tile([128, 4, 8, N_AGENTS], F32R)  # [in%128, ki, b, a]
                for ki in range(4):
                    nc.sync.dma_start(
                        out=xta_sb[:, ki, :, :],
                        in_=dram_ap(
                            xta_d,
                            ki * 128 * b_core * N_AGENTS + b0 * N_AGENTS,
                            [[b_core * N_AGENTS, 128], [N_AGENTS, 8], [1, N_AGENTS]],
                        ),
                    )
                # keep mask, replicated over the 4 head-pair partition groups
                keep_bc = insp.tile([128, 8, NE], F32)  # [(cg,a), b, e]
                for cg in range(4):
                    nc.sync.dma_start(
                        out=keep_bc[cg * 32 : (cg + 1) * 32, :, :],
                        in_=dram_ap(
                            keep_d,
                            b0 * N_AGENTS * NE,
                            [[NE, 32], [N_AGENTS * NE, 8], [1, NE]],
                        ),
                    )
                pkeep_bc = insp.tile([128, 8, N_AGENTS], F32)
                nc.gpsimd.dma_start(
                    out=pkeep_bc,
                    in_=dram_ap(
                        pkeep_d,
                        b0 * N_AGENTS,
                        [[0, 128], [N_AGENTS, 8], [1, N_AGENTS]],
                    ),
                )

                # ---- QKV projections ----
                qt_sb = midp.tile([128, 4, 8, N_AGENTS], BF16)  # [qf%128, qf//128, b, a]
                for mo in range(4):
                    q_ps = ps_mm.tile([128, 8, N_AGENTS], F32, tag="mm")
                    for ki in range(4):
                        nc.tensor.matmul(
                            q_ps,
                            wi_sb[:, ki, mo * 128 : (mo + 1) * 128],
                            xta_sb[:, ki, :, :],
                            start=(ki == 0),
                            stop=(ki == 3),
                        )
                    nc.vector.tensor_copy(out=qt_sb[:, mo, :, :], in_=q_ps)

                kt_sb = midp.tile([128, 4, 8, NE], BF16)  # [kf%128, kf//128, b, e]
                for mo in range(4):
                    for g2 in range(2):
                        k_ps = ps_mm.tile([128, 4, NE], F32, tag="mm")
                        for ki in range(4):
                            nc.tensor.matmul(
                                k_ps,
                                wi_sb[:, ki, EMBED + mo * 128 : EMBED + (mo + 1) * 128],
                                xt_sb[:, ki, g2 * 4 : (g2 + 1) * 4, :],
                                start=(ki == 0),
                                stop=(ki == 3),
                            )
                        nc.scalar.copy(out=kt_sb[:, mo, g2 * 4 : (g2 + 1) * 4, :], in_=k_ps)

                vt_sb = midp.tile([128, 8, EMBED], BF16)  # [e, b, (h,d)]
                for b in range(8):
                    v_ps = ps_mm.tile([128, EMBED], F32, tag="mm")
                    for ki in range(4):
                        nc.tensor.matmul(
                            v_ps,
                            xt_sb[:, ki, b, :],
                            wi_sb[:, ki, 2 * EMBED : 3 * EMBED],
                            start=(ki == 0),
                            stop=(ki == 3),
                        )
                    nc.scalar.copy(out=vt_sb[:, b, :], in_=v_ps)

                # ---- attention, 4 sub-chunks of 2 batches ----
                at_ps = ps_at.tile([128, 8, 4, N_AGENTS], F32)  # [(h%2)*64+d, b, h//2, a]
                for sc in range(4):
                    # separate psum tiles per PE row-half: row tiles writing the
                    # same PSUM bank concurrently is a hardware fault
                    lg = [
                        ps_lg.tile([128, 2, NE], F32, tag="lg0", name="lg0"),
                        ps_lg.tile([128, 2, NE], F32, tag="lg1", name="lg1"),
                    ]  # [(h//2)*32+a, bs, e] for h%2 = 0, 1
                    for bs in range(2):
                        b = sc * 2 + bs
                        for h in range(8):
                            rh, cg = h % 2, h // 2
                            nc.tensor.matmul(
                                lg[rh][cg * 32 : (cg + 1) * 32, bs, :],
                                qt_sb[rh * 64 : rh * 64 + 64, cg, b, :],
                                kt_sb[rh * 64 : rh * 64 + 64, cg, b, :],
                                start=True,
                                stop=True,
                                tile_position=(rh * 64, cg * 32),
                            )
                    we = attnp.tile([128, 2, 2, NE], F32)
                    for rh in range(2):
                        nc.scalar.activation(
                            out=we[:, :, rh, :], in_=lg[rh], func=Exp, scale=0.125
                        )
                    sums = attnp.tile([128, 4], F32)
                    for bs in range(2):
                        b = sc * 2 + bs
                        for rh in range(2):
                            nc.vector.tensor_mul(
                                we[:, bs, rh, :], we[:, bs, rh, :], keep_bc[:, b, :]
                            )
                    nc.vector.reduce_sum(
                        sums, we, axis=mybir.AxisListType.X
                    )
                    nc.vector.tensor_scalar_add(sums, sums, 1e-30)
                    rcp = attnp.tile([128, 4], F32)
                    nc.vector.reciprocal_approx_fast(out=rcp, in_=sums)
                    wn = attnp.tile([128, 2, 2, NE], BF16)
                    for bs in range(2):
                        for rh in range(2):
                            nc.vector.tensor_scalar_mul(
                                wn[:, bs, rh, :],
                                we[:, bs, rh, :],
                                rcp[:, bs * 2 + rh : bs * 2 + rh + 1],
                            )
                    wt_ps = ps_wt.tile([128, 2, 2, NE], BF16)  # [e, bs, rh, (cg,a)]
                    for bs in range(2):
                        for rh in range(2):
                            nc.tensor.transpose(
                                wt_ps[:, bs, rh, :], wn[:, bs, rh, :], ident
                            )
                    wt_sb = attnp.tile([128, 2, 2, NE], BF16)
                    nc.vector.tensor_copy(out=wt_sb, in_=wt_ps)
                    for bs in range(2):
                        b = sc * 2 + bs
                        for h in range(8):
                            rh, cg = h % 2, h // 2
                            nc.tensor.matmul(
                                at_ps[rh * 64 : rh * 64 + 64, b, cg, :],
                                vt_sb[:, b, h * 64 : (h + 1) * 64],
                                wt_sb[:, bs, rh, cg * 32 : (cg + 1) * 32],
                                start=True,
                                stop=True,
                                tile_position=(0, rh * 64),
                            )

                # ---- output projection ----
                attn_sb = outsp.tile([128, 8, 4, N_AGENTS], F32R)
                nc.scalar.copy(out=attn_sb, in_=at_ps)
                out_sb = outsp.tile([128, 4, 8, N_AGENTS], F32)
                for mh in range(2):
                    op_ps = ps_op.tile([128, 2, 8, N_AGENTS], F32)  # [of%128, of//128, b, a]
                    for m2 in range(2):
                        mo2 = mh * 2 + m2
                        for ki2 in range(4):
                            nc.tensor.matmul(
                                op_ps[:, m2, :, :],
                                wo_sb[:, ki2, mo2 * 128 : (mo2 + 1) * 128],
                                attn_sb[:, :, ki2, :],
                                start=(ki2 == 0),
                                stop=(ki2 == 3),
                            )
                    for m2 in range(2):
                        mo2 = mh * 2 + m2
                        nc.vector.tensor_scalar_add(
                            out_sb[:, mo2, :, :],
                            op_ps[:, m2, :, :],
                            bias_sb[:, mo2 : mo2 + 1],
                        )
                        nc.vector.tensor_mul(
                            out_sb[:, mo2, :, :], out_sb[:, mo2, :, :], pkeep_bc
                        )
                nc.sync.dma_start(
                    out=dram_ap(
                        out_d,
                        b0 * N_AGENTS,
                        [[b_core * N_AGENTS, 128],
                         [128 * b_core * N_AGENTS, 4],
                         [N_AGENTS, 8],
                         [1, N_AGENTS]],
                    ),
                    in_=out_sb,
                )

    nc.compile()
    return nc


def _prep_core_inputs(ents, keep, pkeep, wi, wo, bias):
    """Host-side layout prep for one core's batch shard."""
    b_core = ents.shape[0]
    xt = np.ascontiguousarray(ents.transpose(0, 2, 1))  # [b, in, e]
    xta = np.ascontiguousarray(
        ents[:, :N_AGENTS, :].transpose(2, 0, 1)
    ).reshape(4, 128, b_core, N_AGENTS)
    return {
        "xt": xt,
        "xta": xta,
        "wi": wi,
        "wo": wo,
        "keep": keep,
        "pkeep": pkeep,
        "bias": bias,
    }


def run(entities, pre_mask, post_mask, W_in, W_out, b_out, trace=False):
    """Shard, run on 8 cores, gather. Returns (out, BassKernelResults)."""
    from concourse.bass_utils import run_bass_kernel_spmd

    bs = entities.shape[0]
    b_core = bs // N_CORES
    entities = np.asarray(entities, dtype=np.float32)
    keep = (~np.asarray(pre_mask)).astype(np.float32)
    pkeep = (~np.asarray(post_mask)).astype(np.float32)
    wi = np.ascontiguousarray(np.asarray(W_in, dtype=np.float32).T)
    wo = np.ascontiguousarray(np.asarray(W_out, dtype=np.float32).T)
    bias = np.asarray(b_out, dtype=np.float32)

    nc = build_nc(b_core)
    in_maps = [
        _prep_core_inputs(
            entities[c * b_core : (c + 1) * b_core],
            keep[c * b_core : (c + 1) * b_core],
            pkeep[c * b_core : (c + 1) * b_core],
            wi, wo, bias,
        )
        for c in range(N_CORES)
    ]
    res = run_bass_kernel_spmd(nc, in_maps, list(range(N_CORES)), trace=trace)
    out = np.empty((bs, N_AGENTS, OUT_DIM), dtype=np.float32)
    for c in range(N_CORES):
        out[c * b_core : (c + 1) * b_core] = res.results[c]["out"].transpose(1, 2, 0)
    return out, res


def kernel(entities, pre_mask, post_mask, W_in, W_out, b_out):
    out, _ = run(entities, pre_mask, post_mask, W_in, W_out, b_out, trace=False)
    return out


# revision 10
# speedup vs baseline: 1.0159x; 1.0159x over previous
"""EntityAttentionLayer on 8 Trainium2 NeuronCores (Bass/Tile).

Reference computation (per batch b of 1024):
    qkv = entities @ W_in.T            # [128 ents, 3*512]
    q (first 32 ents), k, v -> 8 heads x 64
    logits = q k^T / 8, masked by pre_mask (True = masked out)
    w = softmax(logits), fully-masked rows -> 0
    out = (w v) @ W_out.T + b_out, zeroed where post_mask

Sharding: data-parallel over batch, 128 batches per core.

Per-core kernel layout choices:
  - All big matmuls run as fp32r (TF32-class, full PE rate at N>=256).
  - QKV projections computed feature-major (q^T, k^T: [feat, tok]) for the
    logits matmuls; V computed token-major ([ent, feat]) for attn@v.
  - logits psum tile is [(head//2)*32+agent, batch, head%2, ent] so all 8
    heads of one batch run as one 8-slot tile_position group on the PE.
  - softmax over the free (ent) axis: fused mask-mul+row-sum
    (tensor_tensor_reduce), reciprocal_approx_fast, per-partition scale.
  - w is PE-transposed per (batch, head-parity) so attn@v contracts over
    entities; attn output lands feature-major, feeding the out-projection
    directly; final output is stored [out_feat, batch, agent] and
    untransposed on the host.
"""
import sys

sys.path.insert(0, "/opt/trn_rl_repo")

import numpy as np
import ml_dtypes

BS, NE, IN_DIM = 1024, 128, 512
EMBED, OUT_DIM = 512, 512
N_HEADS, N_AGENTS = 8, 32
HEAD_DIM = EMBED // N_HEADS  # 64
N_CORES = 8


def build_nc(b_core: int):
    """Build the per-core Bass program for b_core batches (b_core % 8 == 0)."""
    import concourse.tile as tile
    from concourse import bacc, mybir
    from concourse.masks import make_identity

    F32 = mybir.dt.float32
    F32R = mybir.dt.float32r
    BF16 = mybir.dt.bfloat16
    Exp = mybir.ActivationFunctionType.Exp
    mult = mybir.AluOpType.mult
    add = mybir.AluOpType.add

    assert b_core % 8 == 0
    n_iter = b_core // 8

    nc = bacc.Bacc("TRN2", target_bir_lowering=False, debug=False)

    # DRAM I/O (per core). Strides in elements, C-order host arrays.
    xt_d = nc.declare_dram_parameter("xt", [b_core, IN_DIM, NE], F32R, isOutput=False)
    xta_d = nc.declare_dram_parameter("xta", [4, 128, b_core, N_AGENTS], F32R, isOutput=False)
    wi_d = nc.declare_dram_parameter("wi", [IN_DIM, 3 * EMBED], F32R, isOutput=False)
    wo_d = nc.declare_dram_parameter("wo", [EMBED, OUT_DIM], F32R, isOutput=False)
    keep_d = nc.declare_dram_parameter("keep", [b_core, N_AGENTS, NE], BF16, isOutput=False)
    pkeep_d = nc.declare_dram_parameter("pkeep", [b_core, N_AGENTS], F32, isOutput=False)
    bias_d = nc.declare_dram_parameter("bias", [OUT_DIM], F32, isOutput=False)
    out_d = nc.declare_dram_parameter("out", [OUT_DIM, b_core, N_AGENTS], F32, isOutput=True)

    import concourse.bass as bass

    AP = bass.AP

    def dram_ap(handle, offset, ap):
        base = handle[:]
        return AP(tensor=base.tensor, offset=offset, ap=ap)

    with tile.TileContext(nc) as tc:
        with (
            tc.tile_pool(name="const", bufs=1) as constp,
            tc.tile_pool(name="ins", bufs=2) as insp,
            tc.tile_pool(name="mid", bufs=2) as midp,
            tc.tile_pool(name="attn", bufs=2) as attnp,
            tc.tile_pool(name="outs", bufs=2) as outsp,
            tc.tile_pool(name="ps_mm", bufs=2, space="PSUM") as ps_mm,
            tc.tile_pool(name="ps_lg", bufs=1, space="PSUM") as ps_lg,
            tc.tile_pool(name="ps_wt", bufs=1, space="PSUM") as ps_wt,
            tc.tile_pool(name="ps_at", bufs=1, space="PSUM") as ps_at,
            tc.tile_pool(name="ps_op", bufs=1, space="PSUM") as ps_op,
        ):
            # ---- constants (loaded once) ----
            wi_sb = constp.tile([128, 4, 3 * EMBED], F32R)  # [in%128, in//128, feat]
            nc.sync.dma_start(
                out=wi_sb,
                in_=dram_ap(wi_d, 0, [[3 * EMBED, 128], [128 * 3 * EMBED, 4], [1, 3 * EMBED]]),
            )
            wo_sb = constp.tile([128, 4, OUT_DIM], F32R)
            nc.sync.dma_start(
                out=wo_sb,
                in_=dram_ap(wo_d, 0, [[OUT_DIM, 128], [128 * OUT_DIM, 4], [1, OUT_DIM]]),
            )
            bias_sb = constp.tile([128, 4], F32)
            nc.sync.dma_start(out=bias_sb, in_=dram_ap(bias_d, 0, [[1, 128], [128, 4]]))
            ident = constp.tile([128, 128], BF16)
            make_identity(nc, ident)

            for it in range(n_iter):
                b0 = it * 8
                # ---- input DMAs for this iter (8 batches) ----
                xt_sb = insp.tile([128, 4, 8, NE], F32R)  # [in%128, ki, b, e]
                for ki in range(4):
                    nc.sync.dma_start(
                        out=xt_sb[:, ki, :, :],
                        in_=dram_ap(
                            xt_d,
                            b0 * IN_DIM * NE + ki * 128 * NE,
                            [[NE, 128], [IN_DIM * NE, 8], [1, NE]],
                        ),
                    )
                xta_sb = insp.tile([128, 4, 8, N_AGENTS], F32R)  # [in%128, ki, b, a]
                for ki in range(4):
                    nc.sync.dma_start(
                        out=xta_sb[:, ki, :, :],
                        in_=dram_ap(
                            xta_d,
                            ki * 128 * b_core * N_AGENTS + b0 * N_AGENTS,
                            [[b_core * N_AGENTS, 128], [N_AGENTS, 8], [1, N_AGENTS]],
                        ),
                    )
                # keep mask, replicated over the 4 head-pair partition groups
                keep_bc = insp.tile([128, 8, NE], BF16)  # [(cg,a), b, e]
                for cg in range(4):
                    nc.sync.dma_start(
                        out=keep_bc[cg * 32 : (cg + 1) * 32, :, :],
                        in_=dram_ap(
                            keep_d,
                            b0 * N_AGENTS * NE,
                            [[NE, 32], [N_AGENTS * NE, 8], [1, NE]],
                        ),
                    )
                pkeep_bc = insp.tile([128, 8, N_AGENTS], F32)
                nc.gpsimd.dma_start(
                    out=pkeep_bc,
                    in_=dram_ap(
                        pkeep_d,
                        b0 * N_AGENTS,
                        [[0, 128], [N_AGENTS, 8], [1, N_AGENTS]],
                    ),
                )

                # ---- QKV projections ----
                qt_sb = midp.tile([128, 4, 8, N_AGENTS], BF16)  # [qf%128, qf//128, b, a]
                for mo in range(4):
                    q_ps = ps_mm.tile([128, 8, N_AGENTS], F32, tag="mm")
                    for ki in range(4):
                        nc.tensor.matmul(
                            q_ps,
                            wi_sb[:, ki, mo * 128 : (mo + 1) * 128],
                            xta_sb[:, ki, :, :],
                            start=(ki == 0),
                            stop=(ki == 3),
                        )
                    nc.vector.tensor_copy(out=qt_sb[:, mo, :, :], in_=q_ps)

                kt_sb = midp.tile([128, 4, 8, NE], BF16)  # [kf%128, kf//128, b, e]
                for mo in range(4):
                    for g2 in range(2):
                        k_ps = ps_mm.tile([128, 4, NE], F32, tag="mm")
                        for ki in range(4):
                            nc.tensor.matmul(
                                k_ps,
                                wi_sb[:, ki, EMBED + mo * 128 : EMBED + (mo + 1) * 128],
                                xt_sb[:, ki, g2 * 4 : (g2 + 1) * 4, :],
                                start=(ki == 0),
                                stop=(ki == 3),
                            )
                        nc.scalar.copy(out=kt_sb[:, mo, g2 * 4 : (g2 + 1) * 4, :], in_=k_ps)

                vt_sb = midp.tile([128, 8, EMBED], BF16)  # [e, b, (h,d)]
                for b in range(8):
                    v_ps = ps_mm.tile([128, EMBED], F32, tag="mm")
                    for ki in range(4):
                        nc.tensor.matmul(
                            v_ps,
                            xt_sb[:, ki, b, :],
                            wi_sb[:, ki, 2 * EMBED : 3 * EMBED],
                            start=(ki == 0),
                            stop=(ki == 3),
                        )
                    nc.scalar.copy(out=vt_sb[:, b, :], in_=v_ps)

                # ---- attention, 4 sub-chunks of 2 batches ----
                at_ps = ps_at.tile([128, 8, 4, N_AGENTS], F32)  # [(h%2)*64+d, b, h//2, a]
                for sc in range(4):
                    # separate psum tiles per PE row-half: row tiles writing the
                    # same PSUM bank concurrently is a hardware fault
                    lg = [
                        ps_lg.tile([128, 2, NE], F32, tag="lg0", name="lg0"),
                        ps_lg.tile([128, 2, NE], F32, tag="lg1", name="lg1"),
                    ]  # [(h//2)*32+a, bs, e] for h%2 = 0, 1
                    for bs in range(2):
                        b = sc * 2 + bs
                        for h in range(8):
                            rh, cg = h % 2, h // 2
                            nc.tensor.matmul(
                                lg[rh][cg * 32 : (cg + 1) * 32, bs, :],
                                qt_sb[rh * 64 : rh * 64 + 64, cg, b, :],
                                kt_sb[rh * 64 : rh * 64 + 64, cg, b, :],
                                start=True,
                                stop=True,
                                tile_position=(rh * 64, cg * 32),
                            )
                    we = attnp.tile([128, 2, 2, NE], F32)
                    for rh in range(2):
                        nc.scalar.activation(
                            out=we[:, :, rh, :], in_=lg[rh], func=Exp, scale=0.125
                        )
                    sums = attnp.tile([128, 4], F32)
                    for bs in range(2):
                        b = sc * 2 + bs
                        for rh in range(2):
                            nc.vector.tensor_mul(
                                we[:, bs, rh, :], we[:, bs, rh, :], keep_bc[:, b, :]
                            )
                    nc.vector.reduce_sum(
                        sums, we, axis=mybir.AxisListType.X
                    )
                    nc.vector.tensor_scalar_add(sums, sums, 1e-30)
                    rcp = attnp.tile([128, 4], F32)
                    nc.vector.reciprocal_approx_fast(out=rcp, in_=sums)
                    wn = attnp.tile([128, 2, 2, NE], BF16)
                    for bs in range(2):
                        for rh in range(2):
                            nc.vector.tensor_scalar_mul(
                                wn[:, bs, rh, :],
                                we[:, bs, rh, :],
                                rcp[:, bs * 2 + rh : bs * 2 + rh + 1],
                            )
                    wt_ps = ps_wt.tile([128, 2, 2, NE], BF16)  # [e, bs, rh, (cg,a)]
                    for bs in range(2):
                        for rh in range(2):
                            nc.tensor.transpose(
                                wt_ps[:, bs, rh, :], wn[:, bs, rh, :], ident
                            )
                    wt_sb = attnp.tile([128, 2, 2, NE], BF16)
                    nc.vector.tensor_copy(out=wt_sb, in_=wt_ps)
                    for bs in range(2):
                        b = sc * 2 + bs
                        for h in range(8):
                            rh, cg = h % 2, h // 2
                            nc.tensor.matmul(
                                at_ps[rh * 64 : rh * 64 + 64, b, cg, :],
                                vt_sb[:, b, h * 64 : (h + 1) * 64],
                                wt_sb[:, bs, rh, cg * 32 : (cg + 1) * 32],
                                start=True,
                                stop=True,
                                tile_position=(0, rh * 64),
                            )

                # ---- output projection ----
                attn_sb = outsp.tile([128, 8, 4, N_AGENTS], F32R)
                nc.scalar.copy(out=attn_sb, in_=at_ps)
                out_sb = outsp.tile([128, 4, 8, N_AGENTS], F32)
                for mh in range(2):
                    op_ps = ps_op.tile([128, 2, 8, N_AGENTS], F32)  # [of%128, of//128, b, a]
                    for m2 in range(2):
                        mo2 = mh * 2 + m2
                        for ki2 in range(4):
                            nc.tensor.matmul(
                                op_ps[:, m2, :, :],
                                wo_sb[:, ki2, mo2 * 128 : (mo2 + 1) * 128],
                                attn_sb[:, :, ki2, :],
                                start=(ki2 == 0),
                                stop=(ki2 == 3),
                            )
                    for m2 in range(2):
                        mo2 = mh * 2 + m2
                        nc.vector.tensor_scalar_add(
                            out_sb[:, mo2, :, :],
                            op_ps[:, m2, :, :],
                            bias_sb[:, mo2 : mo2 + 1],
                        )
                        nc.vector.tensor_mul(
                            out_sb[:, mo2, :, :], out_sb[:, mo2, :, :], pkeep_bc
                        )
                nc.sync.dma_start(
                    out=dram_ap(
                        out_d,
                        b0 * N_AGENTS,
                        [[b_core * N_AGENTS, 128],
                         [128 * b_core * N_AGENTS, 4],
                         [N_AGENTS, 8],
                         [1, N_AGENTS]],
                    ),
                    in_=out_sb,
                )

    nc.compile()
    return nc


def _prep_core_inputs(ents, keep, pkeep, wi, wo, bias):
    """Host-side layout prep for one core's batch shard."""
    b_core = ents.shape[0]
    xt = np.ascontiguousarray(ents.transpose(0, 2, 1))  # [b, in, e]
    xta = np.ascontiguousarray(
        ents[:, :N_AGENTS, :].transpose(2, 0, 1)
    ).reshape(4, 128, b_core, N_AGENTS)
    return {
        "xt": xt,
        "xta": xta,
        "wi": wi,
        "wo": wo,
        "keep": keep,
        "pkeep": pkeep,
        "bias": bias,
    }


def run(entities, pre_mask, post_mask, W_in, W_out, b_out, trace=False):
    """Shard, run on 8 cores, gather. Returns (out, BassKernelResults)."""
    from concourse.bass_utils import run_bass_kernel_spmd

    bs = entities.shape[0]
    b_core = bs // N_CORES
    entities = np.asarray(entities, dtype=np.float32)
    keep = (~np.asarray(pre_mask)).astype(ml_dtypes.bfloat16)
    pkeep = (~np.asarray(post_mask)).astype(np.float32)
    wi = np.ascontiguousarray(np.asarray(W_in, dtype=np.float32).T)
    wo = np.ascontiguousarray(np.asarray(W_out, dtype=np.float32).T)
    bias = np.asarray(b_out, dtype=np.float32)

    nc = build_nc(b_core)
    in_maps = [
        _prep_core_inputs(
            entities[c * b_core : (c + 1) * b_core],
            keep[c * b_core : (c + 1) * b_core],
            pkeep[c * b_core : (c + 1) * b_core],
            wi, wo, bias,
        )
        for c in range(N_CORES)
    ]
    res = run_bass_kernel_spmd(nc, in_maps, list(range(N_CORES)), trace=trace)
    out = np.empty((bs, N_AGENTS, OUT_DIM), dtype=np.float32)
    for c in range(N_CORES):
        out[c * b_core : (c + 1) * b_core] = res.results[c]["out"].transpose(1, 2, 0)
    return out, res


def kernel(entities, pre_mask, post_mask, W_in, W_out, b_out):
    out, _ = run(entities, pre_mask, post_mask, W_in, W_out, b_out, trace=False)
    return out


# revision 11
# speedup vs baseline: 1.1865x; 1.1679x over previous
"""EntityAttentionLayer on 8 Trainium2 NeuronCores (Bass/Tile).

Reference computation (per batch b of 1024):
    qkv = entities @ W_in.T            # [128 ents, 3*512]
    q (first 32 ents), k, v -> 8 heads x 64
    logits = q k^T / 8, masked by pre_mask (True = masked out)
    w = softmax(logits), fully-masked rows -> 0
    out = (w v) @ W_out.T + b_out, zeroed where post_mask

Sharding: data-parallel over batch, 128 batches per core.

Per-core kernel design:
  - fp32r (TF32-class) for the QKV and output projections (full PE rate at
    N>=256); bf16 for the attention inner matmuls, which need tile_position
    column packing that 4-byte dtypes don't support.
  - QKV computed feature-major (q^T, k^T: [feat, tok]) feeding the logits
    matmuls directly; V token-major ([ent, feat]) feeding attn@v.
  - logits for all 8 heads of a batch run as one 8-slot tile_position group;
    the two PE row-halves write separate PSUM banks (sharing one bank between
    row tiles is a hardware fault).
  - softmax over the free (ent) axis: mask-multiply, row-sum,
    reciprocal_approx_fast, per-partition scale; fully-masked rows are kept
    finite with a 1e-30 floor so they produce exact zeros like the reference.
  - w is PE-transposed per (batch, head-parity) so attn@v contracts over
    entities; attn lands feature-major, feeding the out-projection directly;
    output is stored [out_feat, batch, agent] and untransposed on the host.
  - The 16 batch-iterations are software-pipelined: iter N's dense QKV
    matmuls are interleaved with iter N-1's attention so the PE never idles
    long enough for the HAM clock-gate to re-throttle it.
"""
import sys

sys.path.insert(0, "/opt/trn_rl_repo")

import numpy as np
import ml_dtypes

BS, NE, IN_DIM = 1024, 128, 512
EMBED, OUT_DIM = 512, 512
N_HEADS, N_AGENTS = 8, 32
HEAD_DIM = EMBED // N_HEADS  # 64
N_CORES = 8


def build_nc(b_core: int):
    """Build the per-core Bass program for b_core batches (b_core % 8 == 0)."""
    import concourse.bass as bass
    import concourse.tile as tile
    from concourse import bacc, mybir
    from concourse.masks import make_identity

    F32 = mybir.dt.float32
    F32R = mybir.dt.float32r
    BF16 = mybir.dt.bfloat16
    Exp = mybir.ActivationFunctionType.Exp

    assert b_core % 8 == 0
    n_iter = b_core // 8

    nc = bacc.Bacc("TRN2", target_bir_lowering=False, debug=False)

    xt_d = nc.declare_dram_parameter("xt", [b_core, IN_DIM, NE], F32R, isOutput=False)
    xta_d = nc.declare_dram_parameter("xta", [4, 128, b_core, N_AGENTS], F32R, isOutput=False)
    wi_d = nc.declare_dram_parameter("wi", [IN_DIM, 3 * EMBED], F32R, isOutput=False)
    wo_d = nc.declare_dram_parameter("wo", [EMBED, OUT_DIM], F32R, isOutput=False)
    keep_d = nc.declare_dram_parameter("keep", [b_core, N_AGENTS, NE], BF16, isOutput=False)
    pkeep_d = nc.declare_dram_parameter("pkeep", [b_core, N_AGENTS], F32, isOutput=False)
    bias_d = nc.declare_dram_parameter("bias", [OUT_DIM], F32, isOutput=False)
    out_d = nc.declare_dram_parameter("out", [OUT_DIM, b_core, N_AGENTS], F32, isOutput=True)

    AP = bass.AP

    def dram_ap(handle, offset, ap):
        base = handle[:]
        return AP(tensor=base.tensor, offset=offset, ap=ap)

    with tile.TileContext(nc) as tc:
        with (
            tc.tile_pool(name="const", bufs=1) as constp,
            tc.tile_pool(name="ins", bufs=2) as insp,
            tc.tile_pool(name="mid", bufs=2) as midp,
            tc.tile_pool(name="attn", bufs=2) as attnp,
            tc.tile_pool(name="outs", bufs=2) as outsp,
            tc.tile_pool(name="ps_mm", bufs=2, space="PSUM") as ps_mm,
            tc.tile_pool(name="ps_lg", bufs=1, space="PSUM") as ps_lg,
            tc.tile_pool(name="ps_wt", bufs=1, space="PSUM") as ps_wt,
            tc.tile_pool(name="ps_at", bufs=1, space="PSUM") as ps_at,
            tc.tile_pool(name="ps_op", bufs=1, space="PSUM") as ps_op,
        ):
            # ---- constants (loaded once) ----
            wi_sb = constp.tile([128, 4, 3 * EMBED], F32R)  # [in%128, in//128, feat]
            nc.sync.dma_start(
                out=wi_sb,
                in_=dram_ap(wi_d, 0, [[3 * EMBED, 128], [128 * 3 * EMBED, 4], [1, 3 * EMBED]]),
            )
            wo_sb = constp.tile([128, 4, OUT_DIM], F32R)
            nc.sync.dma_start(
                out=wo_sb,
                in_=dram_ap(wo_d, 0, [[OUT_DIM, 128], [128 * OUT_DIM, 4], [1, OUT_DIM]]),
            )
            bias_sb = constp.tile([128, 4], F32)
            nc.sync.dma_start(out=bias_sb, in_=dram_ap(bias_d, 0, [[1, 128], [128, 4]]))
            ident = constp.tile([128, 128], BF16)
            make_identity(nc, ident)

            def emit_inputs(it):
                """Issue this iter's input DMAs; returns the state dict."""
                b0 = it * 8
                st = {"it": it}
                st["xt"] = xt_sb = insp.tile([128, 4, 8, NE], F32R, name="xt_sb", tag="xt_sb")
                for ki in range(4):
                    nc.sync.dma_start(
                        out=xt_sb[:, ki, :, :],
                        in_=dram_ap(
                            xt_d,
                            b0 * IN_DIM * NE + ki * 128 * NE,
                            [[NE, 128], [IN_DIM * NE, 8], [1, NE]],
                        ),
                    )
                st["xta"] = xta_sb = insp.tile(
                    [128, 4, 8, N_AGENTS], F32R, name="xta_sb", tag="xta_sb"
                )
                for ki in range(4):
                    nc.sync.dma_start(
                        out=xta_sb[:, ki, :, :],
                        in_=dram_ap(
                            xta_d,
                            ki * 128 * b_core * N_AGENTS + b0 * N_AGENTS,
                            [[b_core * N_AGENTS, 128], [N_AGENTS, 8], [1, N_AGENTS]],
                        ),
                    )
                # keep mask, replicated over the 4 head-pair partition groups
                st["keep"] = keep_bc = insp.tile(
                    [128, 8, NE], BF16, name="keep_bc", tag="keep_bc"
                )
                for cg in range(4):
                    nc.sync.dma_start(
                        out=keep_bc[cg * 32 : (cg + 1) * 32, :, :],
                        in_=dram_ap(
                            keep_d,
                            b0 * N_AGENTS * NE,
                            [[NE, 32], [N_AGENTS * NE, 8], [1, NE]],
                        ),
                    )
                st["pkeep"] = pkeep_bc = insp.tile(
                    [128, 8, N_AGENTS], F32, name="pkeep_bc", tag="pkeep_bc"
                )
                nc.gpsimd.dma_start(
                    out=pkeep_bc,
                    in_=dram_ap(pkeep_d, b0 * N_AGENTS, [[0, 128], [N_AGENTS, 8], [1, N_AGENTS]]),
                )
                st["qt"] = midp.tile([128, 4, 8, N_AGENTS], BF16, name="qt_sb", tag="qt_sb")
                st["kt"] = midp.tile([128, 4, 8, NE], BF16, name="kt_sb", tag="kt_sb")
                st["vt"] = midp.tile([128, 8, EMBED], BF16, name="vt_sb", tag="vt_sb")
                return st

            def emit_q_unit(st, mo):
                q_ps = ps_mm.tile([128, 8, N_AGENTS], F32, tag="mm", name="q_ps")
                for ki in range(4):
                    nc.tensor.matmul(
                        q_ps,
                        wi_sb[:, ki, mo * 128 : (mo + 1) * 128],
                        st["xta"][:, ki, :, :],
                        start=(ki == 0),
                        stop=(ki == 3),
                    )
                nc.vector.tensor_copy(out=st["qt"][:, mo, :, :], in_=q_ps)

            def emit_k_unit(st, mo, g2):
                k_ps = ps_mm.tile([128, 4, NE], F32, tag="mm", name="k_ps")
                for ki in range(4):
                    nc.tensor.matmul(
                        k_ps,
                        wi_sb[:, ki, EMBED + mo * 128 : EMBED + (mo + 1) * 128],
                        st["xt"][:, ki, g2 * 4 : (g2 + 1) * 4, :],
                        start=(ki == 0),
                        stop=(ki == 3),
                    )
                nc.scalar.copy(out=st["kt"][:, mo, g2 * 4 : (g2 + 1) * 4, :], in_=k_ps)

            def emit_v_unit(st, b):
                v_ps = ps_mm.tile([128, EMBED], F32, tag="mm", name="v_ps")
                for ki in range(4):
                    nc.tensor.matmul(
                        v_ps,
                        st["xt"][:, ki, b, :],
                        wi_sb[:, ki, 2 * EMBED : 3 * EMBED],
                        start=(ki == 0),
                        stop=(ki == 3),
                    )
                nc.scalar.copy(out=st["vt"][:, b, :], in_=v_ps)

            def emit_attn_subchunk(st, sc):
                qt_sb, kt_sb, vt_sb = st["qt"], st["kt"], st["vt"]
                if sc == 0:
                    st["at"] = ps_at.tile([128, 8, 4, N_AGENTS], F32, name="at_ps")
                at_ps = st["at"]
                # logits: all 8 heads of a batch as one tile_position group;
                # separate psum tiles per row-half (shared bank = HW fault)
                lg = [
                    ps_lg.tile([128, 2, NE], F32, tag="lg0", name="lg0"),
                    ps_lg.tile([128, 2, NE], F32, tag="lg1", name="lg1"),
                ]  # [(h//2)*32+a, bs, e] for h%2 = 0, 1
                for bs in range(2):
                    b = sc * 2 + bs
                    for h in range(8):
                        rh, cg = h % 2, h // 2
                        nc.tensor.matmul(
                            lg[rh][cg * 32 : (cg + 1) * 32, bs, :],
                            qt_sb[rh * 64 : rh * 64 + 64, cg, b, :],
                            kt_sb[rh * 64 : rh * 64 + 64, cg, b, :],
                            start=True,
                            stop=True,
                            tile_position=(rh * 64, cg * 32),
                        )
                we = attnp.tile([128, 2, 2, NE], F32, name="we", tag="we")
                for rh in range(2):
                    nc.scalar.activation(out=we[:, :, rh, :], in_=lg[rh], func=Exp, scale=0.125)
                sums = attnp.tile([128, 4], F32, name="sums", tag="sums")
                for bs in range(2):
                    b = sc * 2 + bs
                    for rh in range(2):
                        nc.vector.tensor_mul(
                            we[:, bs, rh, :], we[:, bs, rh, :], st["keep"][:, b, :]
                        )
                nc.vector.reduce_sum(sums, we, axis=mybir.AxisListType.X)
                nc.vector.tensor_scalar_add(sums, sums, 1e-30)
                rcp = attnp.tile([128, 4], F32, name="rcp", tag="rcp")
                nc.vector.reciprocal_approx_fast(out=rcp, in_=sums)
                wn = attnp.tile([128, 2, 2, NE], BF16, name="wn", tag="wn")
                for bs in range(2):
                    for rh in range(2):
                        nc.vector.tensor_scalar_mul(
                            wn[:, bs, rh, :],
                            we[:, bs, rh, :],
                            rcp[:, bs * 2 + rh : bs * 2 + rh + 1],
                        )
                wt_ps = ps_wt.tile([128, 2, 2, NE], BF16, name="wt_ps")  # [e, bs, rh, (cg,a)]
                for bs in range(2):
                    for rh in range(2):
                        nc.tensor.transpose(wt_ps[:, bs, rh, :], wn[:, bs, rh, :], ident)
                wt_sb = attnp.tile([128, 2, 2, NE], BF16, name="wt_sb", tag="wt_sb")
                nc.vector.tensor_copy(out=wt_sb, in_=wt_ps)
                for bs in range(2):
                    b = sc * 2 + bs
                    for h in range(8):
                        rh, cg = h % 2, h // 2
                        nc.tensor.matmul(
                            at_ps[rh * 64 : rh * 64 + 64, b, cg, :],
                            vt_sb[:, b, h * 64 : (h + 1) * 64],
                            wt_sb[:, bs, rh, cg * 32 : (cg + 1) * 32],
                            start=True,
                            stop=True,
                            tile_position=(0, rh * 64),
                        )

            def emit_outproj(st):
                b0 = st["it"] * 8
                attn_sb = outsp.tile([128, 8, 4, N_AGENTS], F32R, name="attn_sb", tag="attn_sb")
                nc.scalar.copy(out=attn_sb, in_=st["at"])
                out_sb = outsp.tile([128, 4, 8, N_AGENTS], F32, name="out_sb", tag="out_sb")
                for mh in range(2):
                    op_ps = ps_op.tile([128, 2, 8, N_AGENTS], F32, name="op_ps")
                    for m2 in range(2):
                        mo2 = mh * 2 + m2
                        for ki2 in range(4):
                            nc.tensor.matmul(
                                op_ps[:, m2, :, :],
                                wo_sb[:, ki2, mo2 * 128 : (mo2 + 1) * 128],
                                attn_sb[:, :, ki2, :],
                                start=(ki2 == 0),
                                stop=(ki2 == 3),
                            )
                    for m2 in range(2):
                        mo2 = mh * 2 + m2
                        nc.vector.tensor_scalar_add(
                            out_sb[:, mo2, :, :], op_ps[:, m2, :, :], bias_sb[:, mo2 : mo2 + 1]
                        )
                        nc.vector.tensor_mul(
                            out_sb[:, mo2, :, :], out_sb[:, mo2, :, :], st["pkeep"]
                        )
                nc.sync.dma_start(
                    out=dram_ap(
                        out_d,
                        b0 * N_AGENTS,
                        [[b_core * N_AGENTS, 128],
                         [128 * b_core * N_AGENTS, 4],
                         [N_AGENTS, 8],
                         [1, N_AGENTS]],
                    ),
                    in_=out_sb,
                )

            def qkv_units(st):
                units = []
                for mo in range(4):
                    units.append(lambda mo=mo: emit_q_unit(st, mo))
                for mo in range(4):
                    for g2 in range(2):
                        units.append(lambda mo=mo, g2=g2: emit_k_unit(st, mo, g2))
                for b in range(8):
                    units.append(lambda b=b: emit_v_unit(st, b))
                return units

            def attn_units(st):
                units = [lambda sc=sc: emit_attn_subchunk(st, sc) for sc in range(4)]
                units.append(lambda: emit_outproj(st))
                return units

            # software pipeline: interleave iter N's QKV with iter N-1's attention
            prev = None
            for it in range(n_iter):
                st = emit_inputs(it)
                qu = qkv_units(st)
                au = attn_units(prev) if prev is not None else []
                for i, u in enumerate(qu):
                    u()
                    if i % 4 == 3 and au:
                        au.pop(0)()
                for u in au:
                    u()
                prev = st
            for u in attn_units(prev):
                u()

    nc.compile()
    return nc


def _prep_core_inputs(ents, keep, pkeep, wi, wo, bias):
    """Host-side layout prep for one core's batch shard."""
    b_core = ents.shape[0]
    xt = np.ascontiguousarray(ents.transpose(0, 2, 1))  # [b, in, e]
    xta = np.ascontiguousarray(
        ents[:, :N_AGENTS, :].transpose(2, 0, 1)
    ).reshape(4, 128, b_core, N_AGENTS)
    return {
        "xt": xt,
        "xta": xta,
        "wi": wi,
        "wo": wo,
        "keep": keep,
        "pkeep": pkeep,
        "bias": bias,
    }


def run(entities, pre_mask, post_mask, W_in, W_out, b_out, trace=False):
    """Shard, run on 8 cores, gather. Returns (out, BassKernelResults)."""
    from concourse.bass_utils import run_bass_kernel_spmd

    bs = entities.shape[0]
    b_core = bs // N_CORES
    entities = np.asarray(entities, dtype=np.float32)
    keep = (~np.asarray(pre_mask)).astype(ml_dtypes.bfloat16)
    pkeep = (~np.asarray(post_mask)).astype(np.float32)
    wi = np.ascontiguousarray(np.asarray(W_in, dtype=np.float32).T)
    wo = np.ascontiguousarray(np.asarray(W_out, dtype=np.float32).T)
    bias = np.asarray(b_out, dtype=np.float32)

    nc = build_nc(b_core)
    in_maps = [
        _prep_core_inputs(
            entities[c * b_core : (c + 1) * b_core],
            keep[c * b_core : (c + 1) * b_core],
            pkeep[c * b_core : (c + 1) * b_core],
            wi, wo, bias,
        )
        for c in range(N_CORES)
    ]
    res = run_bass_kernel_spmd(nc, in_maps, list(range(N_CORES)), trace=trace)
    out = np.empty((bs, N_AGENTS, OUT_DIM), dtype=np.float32)
    for c in range(N_CORES):
        out[c * b_core : (c + 1) * b_core] = res.results[c]["out"].transpose(1, 2, 0)
    return out, res


def kernel(entities, pre_mask, post_mask, W_in, W_out, b_out):
    out, _ = run(entities, pre_mask, post_mask, W_in, W_out, b_out, trace=False)
    return out


# revision 12
# speedup vs baseline: 1.2401x; 1.0452x over previous
"""EntityAttentionLayer on 8 Trainium2 NeuronCores (Bass/Tile).

Reference computation (per batch b of 1024):
    qkv = entities @ W_in.T            # [128 ents, 3*512]
    q (first 32 ents), k, v -> 8 heads x 64
    logits = q k^T / 8, masked by pre_mask (True = masked out)
    w = softmax(logits), fully-masked rows -> 0
    out = (w v) @ W_out.T + b_out, zeroed where post_mask

Sharding: data-parallel over batch, 128 batches per core.

Per-core kernel design:
  - fp32r (TF32-class) for the QKV and output projections (full PE rate at
    N>=256); bf16 for the attention inner matmuls, which need tile_position
    column packing that 4-byte dtypes don't support.
  - QKV computed feature-major (q^T, k^T: [feat, tok]) feeding the logits
    matmuls directly; V token-major ([ent, feat]) feeding attn@v.
  - logits for all 8 heads of a batch run as one 8-slot tile_position group;
    the two PE row-halves write separate PSUM banks (sharing one bank between
    row tiles is a hardware fault).
  - softmax over the free (ent) axis: mask-multiply, row-sum,
    reciprocal_approx_fast, per-partition scale; fully-masked rows are kept
    finite with a 1e-30 floor so they produce exact zeros like the reference.
  - w is PE-transposed per (batch, head-parity) so attn@v contracts over
    entities; attn lands feature-major, feeding the out-projection directly;
    output is stored [out_feat, batch, agent] and untransposed on the host.
  - The 16 batch-iterations are software-pipelined: iter N's dense QKV
    matmuls are interleaved with iter N-1's attention so the PE never idles
    long enough for the HAM clock-gate to re-throttle it.
"""
import sys

sys.path.insert(0, "/opt/trn_rl_repo")

import numpy as np
import ml_dtypes

BS, NE, IN_DIM = 1024, 128, 512
EMBED, OUT_DIM = 512, 512
N_HEADS, N_AGENTS = 8, 32
HEAD_DIM = EMBED // N_HEADS  # 64
N_CORES = 8


def build_nc(b_core: int):
    """Build the per-core Bass program for b_core batches (b_core % 8 == 0)."""
    import concourse.bass as bass
    import concourse.tile as tile
    from concourse import bacc, mybir
    from concourse.masks import make_identity

    F32 = mybir.dt.float32
    F32R = mybir.dt.float32r
    BF16 = mybir.dt.bfloat16
    Exp = mybir.ActivationFunctionType.Exp

    assert b_core % 8 == 0
    n_iter = b_core // 8

    nc = bacc.Bacc("TRN2", target_bir_lowering=False, debug=False)

    xt_d = nc.declare_dram_parameter("xt", [b_core, IN_DIM, NE], F32R, isOutput=False)
    xta_d = nc.declare_dram_parameter("xta", [4, 128, b_core, N_AGENTS], F32R, isOutput=False)
    wi_d = nc.declare_dram_parameter("wi", [IN_DIM, 3 * EMBED], F32R, isOutput=False)
    wo_d = nc.declare_dram_parameter("wo", [EMBED, OUT_DIM], F32R, isOutput=False)
    keep_d = nc.declare_dram_parameter("keep", [b_core, N_AGENTS, NE], BF16, isOutput=False)
    pkeep_d = nc.declare_dram_parameter("pkeep", [b_core, N_AGENTS], F32, isOutput=False)
    bias_d = nc.declare_dram_parameter("bias", [OUT_DIM], F32, isOutput=False)
    out_d = nc.declare_dram_parameter("out", [OUT_DIM, b_core, N_AGENTS], F32, isOutput=True)

    AP = bass.AP

    def dram_ap(handle, offset, ap):
        base = handle[:]
        return AP(tensor=base.tensor, offset=offset, ap=ap)

    with tile.TileContext(nc) as tc:
        with (
            tc.tile_pool(name="const", bufs=1) as constp,
            tc.tile_pool(name="ins", bufs=2) as insp,
            tc.tile_pool(name="mid", bufs=2) as midp,
            tc.tile_pool(name="attn", bufs=2) as attnp,
            tc.tile_pool(name="outs", bufs=2) as outsp,
            tc.tile_pool(name="ps_mm", bufs=2, space="PSUM") as ps_mm,
            tc.tile_pool(name="ps_lg", bufs=1, space="PSUM") as ps_lg,
            tc.tile_pool(name="ps_wt", bufs=1, space="PSUM") as ps_wt,
            tc.tile_pool(name="ps_at", bufs=1, space="PSUM") as ps_at,
            tc.tile_pool(name="ps_op", bufs=1, space="PSUM") as ps_op,
        ):
            # ---- constants (loaded once) ----
            wi_sb = []
            for ki in range(4):
                w_ki = constp.tile([128, 3 * EMBED], F32R, name=f"wi_{ki}", tag=f"wi_{ki}")
                nc.sync.dma_start(
                    out=w_ki,
                    in_=dram_ap(wi_d, ki * 128 * 3 * EMBED, [[3 * EMBED, 128], [1, 3 * EMBED]]),
                )
                wi_sb.append(w_ki)
            wo_sb = constp.tile([128, 4, OUT_DIM], F32R)
            nc.sync.dma_start(
                out=wo_sb,
                in_=dram_ap(wo_d, 0, [[OUT_DIM, 128], [128 * OUT_DIM, 4], [1, OUT_DIM]]),
            )
            bias_sb = constp.tile([128, 4], F32)
            nc.sync.dma_start(out=bias_sb, in_=dram_ap(bias_d, 0, [[1, 128], [128, 4]]))
            ident = constp.tile([128, 128], BF16)
            make_identity(nc, ident)

            def emit_inputs(it):
                """Issue this iter's input DMAs; returns the state dict."""
                b0 = it * 8
                st = {"it": it}
                st["xt"] = xt_sb = insp.tile([128, 4, 8, NE], F32R, name="xt_sb", tag="xt_sb")
                for ki in range(4):
                    nc.sync.dma_start(
                        out=xt_sb[:, ki, :, :],
                        in_=dram_ap(
                            xt_d,
                            b0 * IN_DIM * NE + ki * 128 * NE,
                            [[NE, 128], [IN_DIM * NE, 8], [1, NE]],
                        ),
                    )
                st["xta"] = xta_sb = insp.tile(
                    [128, 4, 8, N_AGENTS], F32R, name="xta_sb", tag="xta_sb"
                )
                for ki in range(4):
                    nc.sync.dma_start(
                        out=xta_sb[:, ki, :, :],
                        in_=dram_ap(
                            xta_d,
                            ki * 128 * b_core * N_AGENTS + b0 * N_AGENTS,
                            [[b_core * N_AGENTS, 128], [N_AGENTS, 8], [1, N_AGENTS]],
                        ),
                    )
                # keep mask, replicated over the 4 head-pair partition groups
                st["keep"] = keep_bc = insp.tile(
                    [128, 8, NE], BF16, name="keep_bc", tag="keep_bc"
                )
                for cg in range(4):
                    nc.sync.dma_start(
                        out=keep_bc[cg * 32 : (cg + 1) * 32, :, :],
                        in_=dram_ap(
                            keep_d,
                            b0 * N_AGENTS * NE,
                            [[NE, 32], [N_AGENTS * NE, 8], [1, NE]],
                        ),
                    )
                st["pkeep"] = pkeep_bc = insp.tile(
                    [128, 8, N_AGENTS], F32, name="pkeep_bc", tag="pkeep_bc"
                )
                nc.gpsimd.dma_start(
                    out=pkeep_bc,
                    in_=dram_ap(pkeep_d, b0 * N_AGENTS, [[0, 128], [N_AGENTS, 8], [1, N_AGENTS]]),
                )
                st["qt"] = midp.tile([128, 4, 8, N_AGENTS], BF16, name="qt_sb", tag="qt_sb")
                st["kt"] = midp.tile([128, 4, 8, NE], BF16, name="kt_sb", tag="kt_sb")
                st["vt"] = midp.tile([128, 8, EMBED], BF16, name="vt_sb", tag="vt_sb")
                return st

            def emit_q_unit(st, mo):
                q_ps = ps_mm.tile([128, 8, N_AGENTS], F32, tag="mm", name="q_ps")
                for ki in range(4):
                    nc.tensor.matmul(
                        q_ps,
                        wi_sb[ki][:, mo * 128 : (mo + 1) * 128],
                        st["xta"][:, ki, :, :],
                        start=(ki == 0),
                        stop=(ki == 3),
                    )
                nc.vector.tensor_copy(out=st["qt"][:, mo, :, :], in_=q_ps)

            def emit_k_unit(st, mo, g2):
                k_ps = ps_mm.tile([128, 4, NE], F32, tag="mm", name="k_ps")
                for ki in range(4):
                    nc.tensor.matmul(
                        k_ps,
                        wi_sb[ki][:, EMBED + mo * 128 : EMBED + (mo + 1) * 128],
                        st["xt"][:, ki, g2 * 4 : (g2 + 1) * 4, :],
                        start=(ki == 0),
                        stop=(ki == 3),
                    )
                nc.scalar.copy(out=st["kt"][:, mo, g2 * 4 : (g2 + 1) * 4, :], in_=k_ps)

            def emit_v_unit(st, b):
                v_ps = ps_mm.tile([128, EMBED], F32, tag="mm", name="v_ps")
                for ki in range(4):
                    nc.tensor.matmul(
                        v_ps,
                        st["xt"][:, ki, b, :],
                        wi_sb[ki][:, 2 * EMBED : 3 * EMBED],
                        start=(ki == 0),
                        stop=(ki == 3),
                    )
                nc.scalar.copy(out=st["vt"][:, b, :], in_=v_ps)

            def emit_attn_subchunk(st, sc):
                qt_sb, kt_sb, vt_sb = st["qt"], st["kt"], st["vt"]
                if sc == 0:
                    st["at"] = ps_at.tile([128, 8, 4, N_AGENTS], F32, name="at_ps")
                at_ps = st["at"]
                # logits: all 8 heads of a batch as one tile_position group;
                # separate psum tiles per row-half (shared bank = HW fault)
                lg = [
                    ps_lg.tile([128, 4, NE], F32, tag="lg0", name="lg0"),
                    ps_lg.tile([128, 4, NE], F32, tag="lg1", name="lg1"),
                ]  # [(h//2)*32+a, bs, e] for h%2 = 0, 1
                for bs in range(4):
                    b = sc * 4 + bs
                    for h in range(8):
                        rh, cg = h % 2, h // 2
                        nc.tensor.matmul(
                            lg[rh][cg * 32 : (cg + 1) * 32, bs, :],
                            qt_sb[rh * 64 : rh * 64 + 64, cg, b, :],
                            kt_sb[rh * 64 : rh * 64 + 64, cg, b, :],
                            start=True,
                            stop=True,
                            tile_position=(rh * 64, cg * 32),
                        )
                we = attnp.tile([128, 4, 2, NE], F32, name="we", tag="we")
                for rh in range(2):
                    nc.scalar.activation(out=we[:, :, rh, :], in_=lg[rh], func=Exp, scale=0.125)
                sums = attnp.tile([128, 8], F32, name="sums", tag="sums")
                for bs in range(4):
                    b = sc * 4 + bs
                    for rh in range(2):
                        nc.vector.tensor_mul(
                            we[:, bs, rh, :], we[:, bs, rh, :], st["keep"][:, b, :]
                        )
                nc.vector.reduce_sum(sums, we, axis=mybir.AxisListType.X)
                nc.vector.tensor_scalar_add(sums, sums, 1e-30)
                rcp = attnp.tile([128, 8], F32, name="rcp", tag="rcp")
                nc.vector.reciprocal_approx_fast(out=rcp, in_=sums)
                wn = attnp.tile([128, 4, 2, NE], BF16, name="wn", tag="wn")
                for bs in range(4):
                    for rh in range(2):
                        nc.vector.tensor_scalar_mul(
                            wn[:, bs, rh, :],
                            we[:, bs, rh, :],
                            rcp[:, bs * 2 + rh : bs * 2 + rh + 1],
                        )
                wt_ps = ps_wt.tile([128, 4, 2, NE], BF16, name="wt_ps")  # [e, bs, rh, (cg,a)]
                for bs in range(4):
                    for rh in range(2):
                        nc.tensor.transpose(wt_ps[:, bs, rh, :], wn[:, bs, rh, :], ident)
                wt_sb = attnp.tile([128, 4, 2, NE], BF16, name="wt_sb", tag="wt_sb")
                nc.vector.tensor_copy(out=wt_sb, in_=wt_ps)
                for bs in range(4):
                    b = sc * 4 + bs
                    for h in range(8):
                        rh, cg = h % 2, h // 2
                        nc.tensor.matmul(
                            at_ps[rh * 64 : rh * 64 + 64, b, cg, :],
                            vt_sb[:, b, h * 64 : (h + 1) * 64],
                            wt_sb[:, bs, rh, cg * 32 : (cg + 1) * 32],
                            start=True,
                            stop=True,
                            tile_position=(0, rh * 64),
                        )

            def emit_outproj(st):
                b0 = st["it"] * 8
                attn_sb = outsp.tile([128, 8, 4, N_AGENTS], F32R, name="attn_sb", tag="attn_sb")
                nc.scalar.copy(out=attn_sb, in_=st["at"])
                out_sb = outsp.tile([128, 4, 8, N_AGENTS], F32, name="out_sb", tag="out_sb")
                for mh in range(2):
                    op_ps = ps_op.tile([128, 2, 8, N_AGENTS], F32, name="op_ps")
                    for m2 in range(2):
                        mo2 = mh * 2 + m2
                        for ki2 in range(4):
                            nc.tensor.matmul(
                                op_ps[:, m2, :, :],
                                wo_sb[:, ki2, mo2 * 128 : (mo2 + 1) * 128],
                                attn_sb[:, :, ki2, :],
                                start=(ki2 == 0),
                                stop=(ki2 == 3),
                            )
                    for m2 in range(2):
                        mo2 = mh * 2 + m2
                        nc.vector.tensor_scalar_add(
                            out_sb[:, mo2, :, :], op_ps[:, m2, :, :], bias_sb[:, mo2 : mo2 + 1]
                        )
                        nc.vector.tensor_mul(
                            out_sb[:, mo2, :, :], out_sb[:, mo2, :, :], st["pkeep"]
                        )
                nc.sync.dma_start(
                    out=dram_ap(
                        out_d,
                        b0 * N_AGENTS,
                        [[b_core * N_AGENTS, 128],
                         [128 * b_core * N_AGENTS, 4],
                         [N_AGENTS, 8],
                         [1, N_AGENTS]],
                    ),
                    in_=out_sb,
                )

            def qkv_units(st):
                units = []
                for mo in range(4):
                    units.append(lambda mo=mo: emit_q_unit(st, mo))
                for mo in range(4):
                    for g2 in range(2):
                        units.append(lambda mo=mo, g2=g2: emit_k_unit(st, mo, g2))
                for b in range(8):
                    units.append(lambda b=b: emit_v_unit(st, b))
                return units

            def attn_units(st):
                units = [lambda sc=sc: emit_attn_subchunk(st, sc) for sc in range(2)]
                units.append(lambda: emit_outproj(st))
                return units

            # software pipeline: interleave iter N's QKV with iter N-1's attention
            prev = None
            for it in range(n_iter):
                st = emit_inputs(it)
                qu = qkv_units(st)
                au = attn_units(prev) if prev is not None else []
                for i, u in enumerate(qu):
                    u()
                    if i % 7 == 6 and au:
                        au.pop(0)()
                for u in au:
                    u()
                prev = st
            for u in attn_units(prev):
                u()

    nc.compile()
    return nc


def _prep_core_inputs(ents, keep, pkeep, wi, wo, bias):
    """Host-side layout prep for one core's batch shard."""
    b_core = ents.shape[0]
    xt = np.ascontiguousarray(ents.transpose(0, 2, 1))  # [b, in, e]
    xta = np.ascontiguousarray(
        ents[:, :N_AGENTS, :].transpose(2, 0, 1)
    ).reshape(4, 128, b_core, N_AGENTS)
    return {
        "xt": xt,
        "xta": xta,
        "wi": wi,
        "wo": wo,
        "keep": keep,
        "pkeep": pkeep,
        "bias": bias,
    }


def run(entities, pre_mask, post_mask, W_in, W_out, b_out, trace=False):
    """Shard, run on 8 cores, gather. Returns (out, BassKernelResults)."""
    from concourse.bass_utils import run_bass_kernel_spmd

    bs = entities.shape[0]
    b_core = bs // N_CORES
    entities = np.asarray(entities, dtype=np.float32)
    keep = (~np.asarray(pre_mask)).astype(ml_dtypes.bfloat16)
    pkeep = (~np.asarray(post_mask)).astype(np.float32)
    wi = np.ascontiguousarray(np.asarray(W_in, dtype=np.float32).T)
    wo = np.ascontiguousarray(np.asarray(W_out, dtype=np.float32).T)
    bias = np.asarray(b_out, dtype=np.float32)

    nc = build_nc(b_core)
    in_maps = [
        _prep_core_inputs(
            entities[c * b_core : (c + 1) * b_core],
            keep[c * b_core : (c + 1) * b_core],
            pkeep[c * b_core : (c + 1) * b_core],
            wi, wo, bias,
        )
        for c in range(N_CORES)
    ]
    res = run_bass_kernel_spmd(nc, in_maps, list(range(N_CORES)), trace=trace)
    out = np.empty((bs, N_AGENTS, OUT_DIM), dtype=np.float32)
    for c in range(N_CORES):
        out[c * b_core : (c + 1) * b_core] = res.results[c]["out"].transpose(1, 2, 0)
    return out, res


def kernel(entities, pre_mask, post_mask, W_in, W_out, b_out):
    out, _ = run(entities, pre_mask, post_mask, W_in, W_out, b_out, trace=False)
    return out
